# revision 41
# baseline (speedup 1.0000x reference)
"""Trainium2 Bass kernel for nn_CIFARDiffusionLayer (5394478923805).

The reference module is LINEAR in u:
  - every tridiagonal ADI solve has batch-independent coefficients
    (built from the tiny [C,32,32] parameter maps), and
  - einsum('cc,bchw->bchw', coupling, u) with the repeated index is a
    per-channel diagonal scale.
So the whole 4-step loop collapses, per channel, to one dense [1024,1024]
matrix L_c acting on flattened 32x32 images:  out[b,c] = L_c @ vec(u[b,c]).
L_c is built on host in float64 by pushing the 1024 basis vectors through the
exact reference recurrences (including the EPS fudge).  Coupling decays fast
with pixel row distance, so per 128-row source chunk only a contiguous window
of output columns carries weight: the device keeps, per (channel, chunk), the
minimal column range covering all |L| >= TAU entries (TAU=1e-6: banding
contributes ~7.6e-3 of the 2e-2 budget; 1.5e-6 already fails the gate) and
runs a banded block matmul — a single data-parallel pass over u.

Numerics (measured rel err 1.02e-2 vs the 2e-2 gate): fp16 input, fp16 W,
fp32 PSUM, INT8 OUTPUT.  The int8 scale 127/S_out is folded into W (the
operator is linear, so the fold is exact); S_out = 1.35x the absmax of a
256-batch host-side sample of L@u.  Linear int8 beats fp8 decisively for a
max-abs-error gate (step = absmax/127 ~ 0.5% vs fp8's 6.25%-of-element).
PSUM then holds int8 target values directly, so drains are plain fp32->int8
casts at the same PSUM-read-bound 690ns as fp16 drains, and stores move
HALF the bytes.  Input-side int8 is a DEAD END, twice measured: casting
DMAs are charged the widened fp16-side bytes (SDMA cost = max side), and
on-chip int8->fp16 widening runs at 20 (DVE) / 36 (GPS) / 107 (ACT)
Gelem/s — below the ~130 Gelem/s the pipeline needs.

Per 128-batch tile (per core, batch-sharded 8 ways): three contiguous
256KB per-channel loads of the tile's channel-blocked pixel-major fp16
block (host does the batch<->pixel transpose while sharding; per-channel
pieces let channel c's matmuls start as soon as its own piece lands) ->
fp16 banded matmuls (fp32 PSUM, 7-deep PSUM rotation), data stationary /
operator moving -> ACT/DVE drains straight to int8 SBUF -> ONE contiguous
384KB int8 store per tile on the gpsimd SWDGE queue.

Schedule lessons baked in (measured on HW, 115us -> 82us -> 74us):
  - The kernel is PE-paced end to end (~3.04us per tile; PE floor 2.96us);
    stores issue at exactly that cadence.  DMA totals 20.7 MB/core at
    ~410-430 GB/s — comfortably off the critical path since int8 out.
  - ALL input traffic (W + u) rides the ONE in-order sync HWDGE queue in
    hand-ordered priority (W_c0h0, u0_c0, W_c0h1, ...): the FIFO gives
    startup-critical pieces the full line rate instead of a 50/50 split
    across queues, and dodges the ~1.3us ACT_TABLE_LOAD stall that delays
    the scalar queue's first dispatch.  Output stores get their own queue
    (gpsimd SWDGE): a store's drain-sem wait must never block a load.
  - Whole-tile contiguous stores only: per-channel final stores are
    row-strided HBM writes whose 2KB descriptors collapse onto ONE SDMA
    engine at ~26 GB/s (a measured 5.4us tail); a lone HWDGE store on an
    idle queue gets a similarly skewed engine spread.
  - The PE p-state ramp is TIME-based (~16us of sustained activity to
    full clock): starting real matmuls earlier just runs more of them at
    low clock.  A short warmup-transpose chain (5) bridges preamble-end
    to first-operand-landing; the chain blocks the first real matmul, so
    it must end exactly when operands arrive.
  - Fixed per-run overheads this path cannot remove: ~6.1us engine
    preamble (excluded from exec_time), ~2.2us final-store HBM receipt +
    teardown barrier, ~6.2us walrus NEFF epilogue that resets the entire
    253-semaphore file one EVENT_SEMAPHORE at a time (~51/engine, gated
    on the final barrier; unconditional regardless of sems actually used),
    and intermittent DVFS throttling (~10-18us windows at 50% DMA cap).
"""
import os
from contextlib import ExitStack

import numpy as np

DT = 0.15
DX = 1.0
NUM_STEPS = 4
EPS = 1e-6
S = 32
C = 3
PIX = S * S          # 1024
KC = PIX // 128      # 8 k-chunks per channel
ROW = C * PIX        # 3072 floats per batch
B_TOTAL = 16384
N_CORES = 8
B_CORE = B_TOTAL // N_CORES
TAU = float(os.environ.get("KERNEL_TAU", "1e-6"))  # operator band threshold

_CACHE = {}
LAST_RESULTS = None  # BassKernelResults of the most recent run (for test.py)


# ----------------------------- host-side operator ---------------------------

def _smooth3(m, axis):
    p = np.concatenate([m.take([0], axis=axis), m, m.take([-1], axis=axis)],
                       axis=axis)
    n = m.shape[axis]
    sl = lambda i: p.take(range(i, i + n), axis=axis)
    return (sl(0) + sl(1) + sl(2)) / 3.0


def _thomas_matrix(a, b, c):
    """Exact linear map of the reference thomas() for one N-system, as [N,N]."""
    N = a.shape[0]
    d = np.eye(N, dtype=np.float64)
    cp = 0.0
    dp = np.zeros(N, dtype=np.float64)
    cs = np.zeros(N, dtype=np.float64)
    ds = np.zeros((N, N), dtype=np.float64)
    for i in range(N):
        denom = b[i] - a[i] * cp + EPS
        cn = c[i] / denom
        dn = (d[i] - a[i] * dp) / denom
        cs[i] = cn
        ds[i] = dn
        cp, dp = cn, dn
    cs[N - 1] = 0.0
    x = np.zeros((N, N), dtype=np.float64)
    xn = np.zeros(N, dtype=np.float64)
    for i in range(N - 1, -1, -1):
        x[i] = ds[i] - cs[i] * xn
        xn = x[i]
    return x


def _solve_matrices(coeff_smooth, dt):
    coeff = coeff_smooth * dt / (DX ** 2)
    a = -coeff
    c = -coeff
    b = 1.0 + 2.0 * coeff
    b = b.copy()
    b[..., 0] = 1.0 + coeff[..., 0]
    b[..., -1] = 1.0 + coeff[..., -1]
    Cn, K, N = a.shape
    out = np.zeros((Cn, K, N, N), dtype=np.float64)
    for ci in range(Cn):
        for k in range(K):
            out[ci, k] = _thomas_matrix(a[ci, k], b[ci, k], c[ci, k])
    return out


def _build_operator(alpha_base, beta_base, alpha_time_coeff, beta_time_coeff,
                    channel_coupling):
    """[C, 1024, 1024] float64: out_vec = L[c] @ u_vec (h*32+w order)."""
    ab = alpha_base.astype(np.float64)
    bb = beta_base.astype(np.float64)
    at = alpha_time_coeff.astype(np.float64)
    bt = beta_time_coeff.astype(np.float64)
    diag = np.diag(channel_coupling.astype(np.float64))

    M = np.broadcast_to(np.eye(PIX, dtype=np.float64).reshape(S, S, PIX),
                        (C, S, S, PIX)).copy()
    t = 0.0
    for _ in range(NUM_STEPS):
        alpha = np.maximum(ab + at * t, EPS)
        beta = np.maximum(bb + bt * t, EPS)
        Sx = _solve_matrices(_smooth3(alpha, axis=2), DT / 2)        # [C,H,w',w]
        bsm = _smooth3(beta, axis=1)
        Sy = _solve_matrices(np.transpose(bsm, (0, 2, 1)), DT)       # [C,W,h',h]
        M = np.einsum('chvw,chwK->chvK', Sx, M)
        M = np.einsum('cwuh,chwK->cuwK', Sy, M)
        M = np.einsum('chvw,chwK->chvK', Sx, M)
        M = M * diag[:, None, None, None]
        t += DT
    return M.reshape(C, PIX, PIX)


def _compute_slices(LT):
    """Per (c, half): [(k, cs, ce, off)] — in-band matmul slices.

    LT: [C, src_pix, out_pix] f64.  Per (c, k-chunk) the minimal contiguous
    out-column range covering every |entry| >= TAU (8-aligned), intersected
    with each 512-col output half.  `off` is the slice's column offset in the
    flat packed W buffer (per channel), assigned in emission order.
    """
    sl = [[[] for _ in range(2)] for _ in range(C)]
    wtot = [[0, 0] for _ in range(C)]
    for c in range(C):
        for h in range(2):
            off = 0
            for k in range(KC):
                M = np.abs(LT[c, k * 128:(k + 1) * 128, :]).max(axis=0)
                idx = np.nonzero(M >= TAU)[0]
                lo = (int(idx[0]) // 8) * 8
                hi = min(-(-int(idx[-1] + 1) // 8) * 8, PIX)
                cs = max(lo, 512 * h) - 512 * h
                ce = min(hi, 512 * h + 512) - 512 * h
                if ce <= cs:
                    continue
                sl[c][h].append((k, cs, ce, off))
                off += ce - cs
            wtot[c][h] = off
    return sl, wtot


# ----------------------------- device program -------------------------------

def _build_program(nc, u_ap, w_aps, id_ap, out_ap, b_per_core, slices):
    import concourse.tile as tile
    from concourse import mybir
    F32 = mybir.dt.float32
    F16 = mybir.dt.float16
    I8 = mybir.dt.int8
    ntiles = b_per_core // 128

    with tile.TileContext(nc) as tc, ExitStack() as ctx:
        const_pool = ctx.enter_context(tc.tile_pool(name="const", bufs=1))
        w_pool = ctx.enter_context(tc.tile_pool(name="w", bufs=1))
        # 14 input-tile buffers (84KB/partition with the other pools, well
        # under the 208KB budget): loads run ahead at full line rate while
        # the PE is still ramp-slow, building an ~8-tile cushion that rides
        # out the intermittent 50%-cap DMA throttle windows (with 10 bufs
        # those windows starved the PE mid-kernel: 3.4-4.2us cadence blips
        # vs the 3.04us steady state, ~2us per run).
        ut_pool = ctx.enter_context(tc.tile_pool(name="ut", bufs=14))
        out_pool = ctx.enter_context(tc.tile_pool(name="out", bufs=8))
        pst_pool = ctx.enter_context(tc.tile_pool(name="pst", bufs=1,
                                                  space="PSUM"))
        psm_pool = ctx.enter_context(tc.tile_pool(name="psm", bufs=7,
                                                  space="PSUM"))

        # Queue assignment decouples the three traffic classes so one class's
        # semaphore wait can never starve another's issue (the HWDGE queues
        # are in-order):
        #   scalar (ACT): operator W (prologue-only) + h=0 drains
        #   sync   (SP):  the 16 fp16 u input loads, nothing else
        #   gpsimd (Pool, SWDGE): the 16 int8 output stores
        # The OUTPUT rides HBM as int8: the quantization scale 127/S_out is
        # folded into W (the operator is linear), so PSUM already holds the
        # int8 target values and the drains are plain fp32->int8 casts --
        # same PSUM-read-bound cost as the old fp16 drains, but the stores
        # move HALF the bytes.  (Input-side int8 was tried and is a dead
        # end: casting DMAs are charged the widened fp16-side bytes, and
        # on-chip int8->fp16 widening runs at 20-107 Gelem/s, far below the
        # 130 Gelem/s the pipeline needs.)
        # u arrives pre-transposed from the host: u_ap[tile, kk, blk*128+b]
        # (pixel-major per 128-batch tile), so each tile is ONE contiguous DMA
        # straight into the matmul operand layout - no PE transposes needed.

        # HAM warm-up: the PE p-state ramp needs ~12-18us of SUSTAINED
        # activity before the clock-gate opens to full 2.4 GHz, so start
        # throwaway transposes the instant the engine preamble ends — gated
        # only on a local DVE memset, not on any DMA — and keep them coming
        # until the first real matmul's operands have landed.
        # 5 chained transposes (~320ns each) bridge the engine-preamble end
        # (~7.6us) to ~9.2us, when the first real operands (W c0h0 + tile0
        # c0, first in the priority-ordered sync queue) have landed.  The
        # warmup chain BLOCKS the first real matmul, so it must end exactly
        # when operands arrive; the p-state ramp then continues through the
        # real matmuls (slower early tiles, but work beats throwaway).
        warm = const_pool.tile([128, 128], F16)
        nc.vector.memset(warm[:], 0.0)
        for wi in range(5):
            wp = pst_pool.tile([128, 128], F16, tag="pst", name="warm")
            nc.tensor.transpose(wp[:], warm[:], warm[:])

        # All input traffic (W + u) rides the ONE in-order sync HWDGE queue
        # in hand-ordered priority: the FIFO guarantees the startup-critical
        # pieces get the full line rate instead of a 50/50 split across two
        # queues, and W avoids the scalar queue where ACT_TABLE_LOAD blocks
        # dispatch for ~1.3us.  First-matmul operands (W c0h0 + tile0 c0,
        # 536KB) land ~9.0us; each later W piece arrives well before its
        # first consumer.  Loads are per channel (contiguous 256KB pieces
        # thanks to the channel-blocked host layout) so channel c's matmuls
        # start as soon as its own piece lands, tracked per-slice by the
        # Tile framework.
        wt = [[None, None] for _ in range(C)]
        u_tiles = {}
        PRO = min(3, ntiles)

        def w_load(c, h):
            t = w_pool.tile([128, w_aps[c][h].shape[-1]], F16,
                            tag=f"w{c}_{h}")
            nc.sync.dma_start(out=t[:], in_=w_aps[c][h])
            wt[c][h] = t

        def load_tile(it, order=(0, 1, 2)):
            u16 = ut_pool.tile([128, ROW], F16, tag="utall", name="utall")
            for cc in order:
                nc.sync.dma_start(
                    out=u16[:, cc * PIX:(cc + 1) * PIX],
                    in_=u_ap[it, cc])
            return u16

        w_load(0, 0)
        u_tiles[0] = ut_pool.tile([128, ROW], F16, tag="utall", name="utall")
        nc.sync.dma_start(out=u_tiles[0][:, 0:PIX], in_=u_ap[0, 0])
        w_load(0, 1)
        nc.sync.dma_start(out=u_tiles[0][:, PIX:2 * PIX], in_=u_ap[0, 1])
        w_load(1, 0)
        w_load(1, 1)
        nc.sync.dma_start(out=u_tiles[0][:, 2 * PIX:3 * PIX], in_=u_ap[0, 2])
        w_load(2, 0)
        w_load(2, 1)
        for it in range(1, PRO):
            u_tiles[it] = load_tile(it)

        def chunks(utall):
            return [[utall[:, (8 * c + k) * 128:(8 * c + k + 1) * 128]
                     for k in range(KC)] for c in range(C)]

        def emit_matmuls(it, ut):
            out_nat = out_pool.tile([128, ROW], I8, name="out_nat")
            for c in range(C):
                for h in range(2):
                    sl = slices[c][h]
                    ps = psm_pool.tile([128, 512], F32, tag="psm", name="ps")
                    for i, (k, cs, ce, off) in enumerate(sl):
                        nc.tensor.matmul(
                            ps[:, cs:ce], lhsT=ut[c][k],
                            rhs=wt[c][h][:, off:off + (ce - cs)],
                            start=(i == 0), stop=(i == len(sl) - 1))
                    # psum drain alternates ACT/DVE so neither engine's
                    # queue gates the PSUM-bank frees
                    if h == 0:
                        nc.scalar.copy(
                            out_nat[:, c * PIX:c * PIX + 512], ps[:])
                    else:
                        nc.vector.tensor_copy(
                            out_nat[:, c * PIX + 512:(c + 1) * PIX], ps[:])
            # ONE contiguous 768KB store per tile on the gpsimd SWDGE
            # queue (rows are full-width slices of out_ap, so the whole
            # tile is one linear HBM region).  (Tried and measured SLOWER:
            # phase-separating reads from writes (+6us), per-channel
            # stores for every tile (+4us) - strided rows collapse onto a
            # single SDMA engine at ~26 GB/s, interleaving tail stores
            # across queues (+14us).)
            nc.gpsimd.dma_start(
                out=out_ap[it * 128:(it + 1) * 128, :], in_=out_nat[:])

        # Prologue: first tiles' matmuls run while W/u still stream in.
        for it in range(PRO):
            emit_matmuls(it, chunks(u_tiles[it]))

        for it in range(PRO, ntiles):
            emit_matmuls(it, chunks(load_tile(it)))


def _get_nc(slices, wtot):
    key = ("nc-i8", str(slices))
    if key in _CACHE:
        return _CACHE[key]
    from concourse import bacc, mybir
    # num_devices=1: the 8 cores are pure SPMD replicas with no collectives,
    # so skip the cross-core EVSEM butterfly in the kernel pre/postamble.
    nd = int(os.environ.get("KERNEL_ND", "1"))
    nc = bacc.Bacc("TRN2", target_bir_lowering=False, debug=False,
                   num_devices=nd)
    F16 = mybir.dt.float16
    I8 = mybir.dt.int8
    u_ap = nc.dram_tensor("u", [B_CORE // 128, C, 128, PIX], F16,
                          kind="ExternalInput").ap()
    w_aps = [[nc.dram_tensor(f"w{c}_{h}", [128, wtot[c][h]], F16,
                             kind="ExternalInput").ap() for h in range(2)]
             for c in range(C)]
    id_ap = nc.dram_tensor("ident", [128, 128], F16,
                           kind="ExternalInput").ap()
    out_ap = nc.dram_tensor("out", [B_CORE, ROW], I8,
                            kind="ExternalOutput").ap()
    _build_program(nc, u_ap, w_aps, id_ap, out_ap, B_CORE, slices)
    nc.compile()
    _CACHE[key] = nc
    return nc


def _inject_ntff_hook():
    import sys, types
    try:
        import antenv.axon_hooks  # noqa: F401
        return
    except ImportError:
        pass
    from trn_agent_boot.trn_boot import _ntff_profile_via_ctypes
    hook = _ntff_profile_via_ctypes('/opt/axon/libaxon_pjrt.so')
    mod = types.ModuleType('antenv.axon_hooks')
    _state = {'hook': hook}
    mod.get_axon_ntff_profile_hook = lambda: _state['hook']
    mod.set_axon_ntff_profile_hook = lambda h: _state.update(hook=h)
    sys.modules['antenv.axon_hooks'] = mod
    import antenv
    antenv.axon_hooks = mod


# ----------------------------- entry point ----------------------------------

def kernel(u, alpha_base, beta_base, alpha_time_coeff, beta_time_coeff,
           channel_coupling):
    global LAST_RESULTS
    u = np.asarray(u, dtype=np.float32)
    assert u.shape == (B_TOTAL, C, S, S), u.shape

    L = _build_operator(np.asarray(alpha_base), np.asarray(beta_base),
                        np.asarray(alpha_time_coeff),
                        np.asarray(beta_time_coeff),
                        np.asarray(channel_coupling))
    LT = L.transpose(0, 2, 1)  # [c, src_pix, out_pix]
    slices, wtot = _compute_slices(LT)

    # The output rides HBM as int8.  Its quantization scale 127/S_out is
    # folded into W (the operator is linear, so this is exact): PSUM then
    # holds the int8 target values directly and the drains are plain
    # fp32->int8 casts.  S_out = 1.35 x the output absmax of a 256-batch
    # host sample (max-ratio between the full 50M-element tensor and the
    # 0.8M-element sample is ~1.15x for Gaussian-ish data, so 1.35 leaves
    # clipping probability negligible while costing only ~0.53% of absmax
    # in quantization step).  Folding also fixes fp16 W range: entries
    # land in [~0.3, ~95], far from both subnormals and overflow.
    uf0 = u.reshape(B_TOTAL, C, PIX)
    samp = np.einsum('cjk,bck->bcj', L,
                     uf0[:256].astype(np.float64))
    S_OUT = 1.35 * float(np.abs(samp).max())
    WSCALE = 127.0 / S_OUT
    ws = []
    for c in range(C):
        wc = []
        for h in range(2):
            w = np.zeros((128, wtot[c][h]), dtype=np.float32)
            for k, cs, ce, off in slices[c][h]:
                w[:, off:off + (ce - cs)] = \
                    LT[c, k * 128:(k + 1) * 128,
                       512 * h + cs:512 * h + ce] * WSCALE
            wc.append(w.astype(np.float16))
        ws.append(wc)
    ident = np.eye(128, dtype=np.float16)

    nc = _get_nc(slices, wtot)
    from concourse import bass_utils

    # channel-blocked pixel-major tiles: u_t[tile, c, p, k*128 + b] — each
    # (tile, c) block is a contiguous 256KB DMA straight into the matmul
    # operand layout (SBUF cols (8c+k)*128+b), and per-channel loads let
    # channel c's matmuls start as soon as its own piece lands.
    u4 = u.reshape(B_TOTAL // 128, 128, C, KC, 128).astype(np.float16)
    u2 = np.ascontiguousarray(u4.transpose(0, 2, 4, 3, 1)).reshape(
        B_TOTAL // 128, C, 128, PIX)
    tpc = B_CORE // 128
    in_maps = [{"u": u2[i * tpc:(i + 1) * tpc], "ident": ident,
                **{f"w{c}_{h}": ws[c][h] for c in range(C) for h in range(2)}}
               for i in range(N_CORES)]

    trace = os.environ.get("KERNEL_TRACE", "") == "1"
    kw = {}
    if trace:
        _inject_ntff_hook()
        bass_utils.upload_artifacts = lambda tmpdir: tmpdir
        kw = dict(trace=True, tmpdir=os.environ.get("KERNEL_TRACE_DIR"))

    # Expected result for one batch row per core, for output verification
    # (the devices occasionally fail transiently — exceptions AND, rarely,
    # silently corrupted buffers — so verify and retry).
    checks = []
    for i in range(N_CORES):
        b = i * B_CORE
        checks.append(np.concatenate(
            [L[c] @ uf0[b, c].astype(np.float64) for c in range(C)]))

    import time
    last_exc = None
    for attempt in range(3):
        if trace and attempt > 0 and kw.get("tmpdir"):
            # stale NTFFs from the failed attempt break the profile
            # conversion (duplicate json paths) — start clean
            import glob as _glob
            for f in _glob.glob(os.path.join(kw["tmpdir"], "*")):
                try:
                    os.remove(f)
                except OSError:
                    pass
        try:
            res = bass_utils.run_bass_kernel_spmd(
                nc, in_maps, core_ids=list(range(N_CORES)), **kw)
        except Exception as e:
            last_exc = e
            time.sleep(5)
            continue
        ok = True
        for i in range(N_CORES):
            got = res.results[i]["out"][0].astype(np.float64) / WSCALE
            ref = checks[i]
            tol = 0.05 * max(np.abs(ref).max(), 1e-30)
            if not np.all(np.isfinite(got)) or np.abs(got - ref).max() > tol:
                ok = False
                break
        if ok:
            break
        time.sleep(5)
    else:
        if last_exc is not None:
            raise last_exc
    LAST_RESULTS = res

    out = np.concatenate([r["out"] for r in res.results], axis=0)
    out = out.astype(np.float32) * (1.0 / WSCALE)
    return out.reshape(B_TOTAL, C, S, S)

